# revision 4
# baseline (speedup 1.0000x reference)
"""Single-head causal attention on 8 TRN2 NeuronCores, batch-parallel.

Problem: x[8,2048,1024] f32, Wq/Wk/Wv[1024,64] f32
  q,k,v = x@W*  ;  scores = q k^T / sqrt(1024), causal  ;  out = softmax(scores) @ v

Sharding: batch dim across 8 cores (1 batch element per core, no collectives).

v2 dataflow (bf16 datapath, DMA-transpose, wave-structured):
  - host passes x/W*/tri in bf16 (tolerance 2e-2 >> bf16's ~4e-3); weights
    pre-packed [c%128, c//128, h] so every DMA is a wide contiguous transfer.
  - xT [c, t] produced DIRECTLY by hardware DMA-transpose (16x128 xbar tiles,
    no PE transposes, no PSUM->SBUF copies), in 4 half-T/quarter-T waves,
    reverse t order so the last-arriving chunk gates the fewest score tiles.
  - per t-tile: qk-proj (one [128,128] psum: rows 0:64 q, 64:128 k -> qkT bf16);
    kT moved to partitions 0:64 via per-chunk SBUF->SBUF DMA; v-proj natural.
  - scores: st[s,t] [128,512] tiles, lhsT=kT rhs=qT (bf16); exp on Act in PAIRS
    ([128,2,512] psum -> wst bf16); diagonal tri-mask on Pool (gpsimd).
  - out natural: po[t,h] = sum_j wst_j^T @ [v_j | 1]; col 64 = softmax denom;
    epilogue reciprocal + tensor_scalar_mul (DVE); out f32 DMA per chunk.
"""

import numpy as np

import concourse.bacc as bacc
import concourse.mybir as mybir
import concourse.tile as tile
from concourse.bass_utils import run_bass_kernel_spmd

F32 = mybir.dt.float32
BF16 = mybir.dt.bfloat16
EXP = mybir.ActivationFunctionType.Exp

B, T, C, H = 8, 2048, 1024, 64
NCT = C // 128          # 8 c-tiles
NTT = T // 128          # 16 t-tiles
SCALE = float(C ** -0.5)

# waves of t-tiles (transposed + projected together), reverse chunk order
WAVES = [[12, 13, 14, 15], [8, 9, 10, 11], [4, 5, 6, 7], [0, 1, 2, 3]]
# st tiles (i, j) released per wave: needs qT(i) and kT(chunk(j))
ST_WAVE = [
    [(3, 12), (3, 13), (3, 14), (3, 15)],
    [(3, 8), (3, 9), (3, 10), (3, 11), (2, 8), (2, 9), (2, 10), (2, 11)],
    [(3, 4), (3, 5), (3, 6), (3, 7), (2, 4), (2, 5), (2, 6), (2, 7),
     (1, 4), (1, 5), (1, 6), (1, 7)],
    [(3, 0), (3, 1), (3, 2), (3, 3), (2, 0), (2, 1), (2, 2), (2, 3),
     (1, 0), (1, 1), (1, 2), (1, 3), (0, 0), (0, 1), (0, 2), (0, 3)],
]
AVAIL = [12, 13, 14, 15, 8, 9, 10, 11, 4, 5, 6, 7, 0, 1, 2, 3]

_CACHE = {}


def build():
    nc = bacc.Bacc(name="head_attn")
    xb_d = nc.dram_tensor("xb", [T, C], BF16, kind="ExternalInput")
    wqk_d = nc.dram_tensor("wqkb", [128, NCT, 128], BF16, kind="ExternalInput")
    wv_d = nc.dram_tensor("wvb", [128, NCT, H], BF16, kind="ExternalInput")
    tri_d = nc.dram_tensor("trib", [128, 128], BF16, kind="ExternalInput")
    out_d = nc.dram_tensor("out", [T, H], F32, kind="ExternalOutput")

    with tile.TileContext(nc) as tc:
        with (
            tc.tile_pool(name="singles", bufs=1) as singles,
            tc.tile_pool(name="wstp", bufs=1) as wstp,
            tc.tile_pool(name="outp", bufs=1) as outp,
            tc.tile_pool(name="recp", bufs=2) as recp,
            tc.tile_pool(name="pqv", bufs=2, space="PSUM") as pqv,
            tc.tile_pool(name="pst", bufs=3, space="PSUM") as pst,
        ):
            # ---- weights / constants (Act hwdge queue, wide contiguous DMAs)
            wqkb = singles.tile([128, NCT, 128], BF16)
            wvb = singles.tile([128, NCT, H], BF16)
            trib = singles.tile([128, 128], BF16)
            nc.scalar.dma_start(wqkb, wqk_d[:, :, :])
            nc.scalar.dma_start(wvb, wv_d[:, :, :])
            nc.scalar.dma_start(trib, tri_d[:, :])

            xT = singles.tile([128, NCT, T], BF16)      # [c, ct, t]
            qkT = singles.tile([128, T], BF16)          # rows 0:64 qT, 64:128 kT
            kTsb = singles.tile([64, T], BF16)          # kT at base partition 0
            v_sb = singles.tile([128, NTT, 66], BF16)   # v natural + ones col 64
            nc.gpsimd.memset(v_sb[:, :, 64:66], 1.0)

            out_v = out_d.rearrange("(c a p) h -> c p a h", a=4, p=128)

            wst_loc = {}
            n_exp = [0]

            def flush_pairs(pairs):
                for g in pairs:
                    pt = pst.tile([128, 2, 512], F32, tag="pair", name="pt")
                    for h, (i, j) in enumerate(g):
                        nc.tensor.matmul(pt[:, h, :],
                                         kTsb[:, j * 128:(j + 1) * 128],
                                         qkT[0:64, i * 512:(i + 1) * 512],
                                         start=True, stop=True)
                    wt = wstp.tile([128, 2, 512], BF16, tag=f"w{n_exp[0]}",
                                   name="wt")
                    n_exp[0] += 1
                    nc.scalar.activation(wt[:, 0:len(g), :], pt[:, 0:len(g), :],
                                         EXP, scale=SCALE)
                    for h, (i, j) in enumerate(g):
                        wst_loc[(i, j)] = (wt, h)
                        k = j - 4 * i
                        if k >= 0:      # diagonal tile: mask lower triangle
                            nc.gpsimd.tensor_mul(
                                wt[:, h, k * 128:(k + 1) * 128],
                                wt[:, h, k * 128:(k + 1) * 128], trib)

            # ---- wave loop
            for w, tiles in enumerate(WAVES):
                t0, t1 = tiles[0] * 128, (tiles[-1] + 1) * 128
                for ct in range(NCT):
                    nc.sync.dma_start_transpose(
                        xT[:, ct, t0:t1], xb_d[t0:t1, ct * 128:(ct + 1) * 128])

                for tt in tiles:
                    pq = pqv.tile([128, 128], F32, tag="pqv", name="pq")
                    for ct in range(NCT):
                        nc.tensor.matmul(pq, wqkb[:, ct, :],
                                         xT[:, ct, tt * 128:(tt + 1) * 128],
                                         start=(ct == 0), stop=(ct == NCT - 1))
                    nc.vector.tensor_copy(qkT[:, tt * 128:(tt + 1) * 128], pq)

                    pv = pqv.tile([128, 128], F32, tag="pqv", name="pv")
                    for ct in range(NCT):
                        nc.tensor.matmul(pv[:, 0:H],
                                         xT[:, ct, tt * 128:(tt + 1) * 128],
                                         wvb[:, ct, :],
                                         start=(ct == 0), stop=(ct == NCT - 1))
                    nc.vector.tensor_copy(v_sb[:, tt, 0:H], pv[:, 0:H])

                # kT rows -> base partition 0 (DMA moves across partitions)
                c = tiles[0] // 4
                nc.sync.dma_start(kTsb[:, c * 512:(c + 1) * 512],
                                  qkT[64:128, c * 512:(c + 1) * 512])

                sts = ST_WAVE[w]
                flush_pairs([sts[n:n + 2] for n in range(0, len(sts), 2)])

            # ---- output: po bursts (gated on the final wave anyway)
            done = {c: 0 for c in range(4)}
            ob = {}
            for c in range(4):
                obt = outp.tile([128, 4, H], F32, tag=f"ob{c}", name=f"ob{c}")
                ob[c] = obt
            for tt in AVAIL:
                i, tl = tt // 4, tt % 4
                js = [j for j in AVAIL if j <= tt]
                pp = pqv.tile([128, 128], F32, tag="pqv", name="pp")
                for n, j in enumerate(js):
                    wt, h = wst_loc[(i, j)]
                    nc.tensor.matmul(pp[:, 0:66],
                                     wt[:, h, tl * 128:(tl + 1) * 128],
                                     v_sb[:, j, 0:66],
                                     start=(n == 0), stop=(n == len(js) - 1))
                rec = recp.tile([128, 1], F32, tag="rec", name="rec")
                nc.vector.reciprocal(rec, pp[:, 64:65])
                nc.vector.tensor_scalar_mul(ob[i][:, tl, :], pp[:, 0:H], rec)
                done[i] += 1
                if done[i] == 4:
                    nc.scalar.dma_start(out_v[i], ob[i])

    nc.compile()
    return nc


def kernel(x, Wq, Wk, Wv, trace=False):
    import ml_dtypes
    BF = ml_dtypes.bfloat16
    x = np.ascontiguousarray(np.asarray(x, dtype=np.float32))
    Wq = np.asarray(Wq, dtype=np.float32)
    Wk = np.asarray(Wk, dtype=np.float32)
    Wv = np.asarray(Wv, dtype=np.float32)

    if "nc" not in _CACHE:
        _CACHE["nc"] = build()
    nc = _CACHE["nc"]

    xb = np.ascontiguousarray(x.astype(BF))                       # [B, T, C]
    wqkb = np.ascontiguousarray(np.concatenate(
        [Wq.reshape(NCT, 128, H), Wk.reshape(NCT, 128, H)],
        axis=-1).transpose(1, 0, 2).astype(BF))                   # [128, 8, 128]
    wvb = np.ascontiguousarray(
        Wv.reshape(NCT, 128, H).transpose(1, 0, 2).astype(BF))    # [128, 8, 64]
    trib = np.triu(np.ones((128, 128), dtype=np.float32)).astype(BF)

    in_maps = [
        {"xb": xb[b], "wqkb": wqkb, "wvb": wvb, "trib": trib}
        for b in range(B)
    ]
    try:
        res = run_bass_kernel_spmd(nc, in_maps, core_ids=list(range(B)), trace=trace)
    except ModuleNotFoundError:
        res = run_bass_kernel_spmd(nc, in_maps, core_ids=list(range(B)))
    out = np.stack([r["out"] for r in res.results], axis=0)
    kernel.last_exec_time_ns = res.exec_time_ns
    kernel.last_results = res
    return out
